# revision 1
# baseline (speedup 1.0000x reference)
import sys
sys.path.insert(0, '/opt/trn_rl_repo')
import numpy as np
import ml_dtypes
import concourse.bacc as bacc
import concourse.mybir as mybir
import concourse.tile as tile
from concourse.bass_utils import run_bass_kernel_spmd

F32 = mybir.dt.float32
BF16 = mybir.dt.bfloat16
ALU = mybir.AluOpType
ACTF = mybir.ActivationFunctionType

B, T, H, O = 16, 2048, 512, 512
NB = 2            # batch rows per core
NCORES = 8
L = 16            # scan chunk length
C = T // L        # chunks per row = 128
NMT = T // 512    # mtiles per row = 4
LN_EPS = 1e-6

_CACHE = {}


def _build():
    nc = bacc.Bacc(None, target_bir_lowering=False)
    xin = nc.declare_dram_parameter("x_t", [NB, H, T], BF16, False)
    Brg = nc.declare_dram_parameter("Brg", [H, H], BF16, False)
    Big = nc.declare_dram_parameter("Big", [H, H], BF16, False)
    Crt = nc.declare_dram_parameter("Crt", [H, H], BF16, False)
    Cin = nc.declare_dram_parameter("Cin", [H, H], BF16, False)
    W1s = nc.declare_dram_parameter("W1s", [H, 4 * H], BF16, False)
    W2t = nc.declare_dram_parameter("W2t", [4 * H, O], BF16, False)
    CST = nc.declare_dram_parameter("cst", [128, 4 * 29], F32, False)
    out = nc.declare_dram_parameter("out_t", [NB, O, T], F32, True)

    with tile.TileContext(nc) as tc:
        with tc.tile_pool(name="wpool", bufs=1) as wp, \
             tc.tile_pool(name="upool", bufs=1) as up, \
             tc.tile_pool(name="xp", bufs=2) as xp, \
             tc.tile_pool(name="sc", bufs=2) as scp, \
             tc.tile_pool(name="yp", bufs=4) as yp, \
             tc.tile_pool(name="p1p", bufs=17) as p1p, \
             tc.tile_pool(name="fp", bufs=2) as fp, \
             tc.tile_pool(name="abp", bufs=2) as abp, \
             tc.tile_pool(name="ps_mm1", bufs=2, space="PSUM") as ps1, \
             tc.tile_pool(name="ps_st", bufs=2, space="PSUM") as pst, \
             tc.tile_pool(name="ps_mm3", bufs=2, space="PSUM") as ps3, \
             tc.tile_pool(name="ps_mm4", bufs=2, space="PSUM") as ps4:
            ps2 = ps1

            # ---- resident weights ----
            brg_t = wp.tile([128, 4 * 512], BF16, tag="brg")
            big_t = wp.tile([128, 4 * 512], BF16, tag="big")
            cr_t = wp.tile([128, 4 * 512], BF16, tag="cr")
            ci_t = wp.tile([128, 4 * 512], BF16, tag="ci")
            w1_t = wp.tile([128, 4 * 2048], BF16, tag="w1")
            w2_t = wp.tile([128, 16 * 512], BF16, tag="w2")
            cst_t = wp.tile([128, 4 * 29], F32, tag="cst")
            ones_t = wp.tile([128, 128], BF16, tag="ones")
            eps_t = wp.tile([128, 1], F32, tag="eps")
            for (dst, src, kk) in ((brg_t, Brg, 4), (big_t, Big, 4),
                                   (cr_t, Crt, 4), (ci_t, Cin, 4),
                                   (w1_t, W1s, 4), (w2_t, W2t, 16)):
                nc.sync.dma_start(
                    out=dst[:].rearrange("p (k n) -> p k n", k=kk),
                    in_=src[:].rearrange("(k p) n -> p k n", p=128))
            nc.sync.dma_start(out=cst_t[:], in_=CST[:])
            nc.vector.memset(ones_t[:], 1.0)
            nc.vector.memset(eps_t[:], LN_EPS)

            def col(c, blk):
                return cst_t[:, c * 4 + blk:c * 4 + blk + 1]

            # ---- U buffers (bf16, in-place scan) ----
            Ur = up.tile([128, 4 * NB * T], BF16, tag="ur")
            Ui = up.tile([128, 4 * NB * T], BF16, tag="ui")
            ur4 = Ur[:].rearrange("p (Bk b j c) -> p Bk b c j", Bk=4, b=NB, j=L, c=C)
            ui4 = Ui[:].rearrange("p (Bk b j c) -> p Bk b c j", Bk=4, b=NB, j=L, c=C)
            Gr = up.tile([128, 4 * NB * (C + 1)], F32, tag="gr")
            Gi = up.tile([128, 4 * NB * (C + 1)], F32, tag="gi")
            g5r = Gr[:].rearrange("p (Bk b s) -> p Bk b s", Bk=4, b=NB)
            g5i = Gi[:].rearrange("p (Bk b s) -> p Bk b s", Bk=4, b=NB)
            nc.vector.memset(Gr[:], 0.0)
            nc.vector.memset(Gi[:], 0.0)

            def ucol(u, blk, b, j):
                base = blk * NB * T + b * T + j * C
                return u[:, base:base + C]

            def gsl(g5, blk, b, s0, s1):
                return g5[:, blk:blk + 1, b:b + 1, s0:s1].squeeze()

            def mm1(b):
                for mt in range(NMT):
                    t0 = mt * 512
                    xt = xp.tile([128, 4 * 512], BF16, tag="xt")
                    nc.sync.dma_start(
                        out=xt[:].rearrange("p (k t) -> p k t", k=4),
                        in_=xin[b, :, t0:t0 + 512].rearrange("(k p) t -> p k t", p=128))
                    for (wt, dst, bcol) in ((brg_t, Ur, 3), (big_t, Ui, 4)):
                        for ob in range(4):
                            pm = ps1.tile([128, 512], F32, tag="pm1")
                            for kt in range(4):
                                nc.tensor.matmul(
                                    pm[:], wt[:, kt * 512 + ob * 128:kt * 512 + ob * 128 + 128],
                                    xt[:, kt * 512:(kt + 1) * 512],
                                    start=(kt == 0), stop=(kt == 3))
                            dv = ur4 if dst is Ur else ui4
                            slab = dv[:, ob:ob + 1, b:b + 1, mt * 32:(mt + 1) * 32, :].squeeze()
                            nc.scalar.activation(
                                slab, pm[:], ACTF.Identity, bias=col(bcol, ob), scale=1.0)

            def scan(b):
                # pass A: state recurrence, E -> G slots 1..C
                for blk in range(4):
                    stR = [scp.tile([128, C], F32, tag="stR0", name="stR0"),
                           scp.tile([128, C], F32, tag="stR1", name="stR1")]
                    stI = [scp.tile([128, C], F32, tag="stI0", name="stI0"),
                           scp.tile([128, C], F32, tag="stI1", name="stI1")]
                    lr, li, nli = col(0, blk), col(1, blk), col(2, blk)
                    for j in range(1, L):
                        pR = ucol(Ur, blk, b, j - 1) if j == 1 else stR[(j - 1) % 2][:]
                        pI = ucol(Ui, blk, b, j - 1) if j == 1 else stI[(j - 1) % 2][:]
                        cR = stR[j % 2][:]
                        cI = stI[j % 2][:]
                        oR = gsl(g5r, blk, b, 1, C + 1) if j == L - 1 else cR
                        oI = gsl(g5i, blk, b, 1, C + 1) if j == L - 1 else cI
                        nc.vector.scalar_tensor_tensor(cR, pI, nli, ucol(Ur, blk, b, j), ALU.mult, ALU.add)
                        nc.vector.scalar_tensor_tensor(oR, pR, lr, cR, ALU.mult, ALU.add)
                        nc.vector.scalar_tensor_tensor(cI, pR, li, ucol(Ui, blk, b, j), ALU.mult, ALU.add)
                        nc.vector.scalar_tensor_tensor(oI, pI, lr, cI, ALU.mult, ALU.add)
                # level-2 Hillis-Steele on G (slots 1..C), p_k = lam^(L*2^k)
                for k in range(7):
                    s = 1 << k
                    for blk in range(4):
                        hr, hi, nhi = col(8 + 3 * k, blk), col(9 + 3 * k, blk), col(10 + 3 * k, blk)
                        src_r = gsl(g5r, blk, b, 1, C + 1 - s)
                        src_i = gsl(g5i, blk, b, 1, C + 1 - s)
                        tgt_r = gsl(g5r, blk, b, 1 + s, C + 1)
                        tgt_i = gsl(g5i, blk, b, 1 + s, C + 1)
                        tR = scp.tile([128, C], F32, tag="tR")
                        tI = scp.tile([128, C], F32, tag="tI")
                        n = C - s
                        nc.vector.scalar_tensor_tensor(tR[:, :n], src_r, hr, tgt_r, ALU.mult, ALU.add)
                        nc.vector.scalar_tensor_tensor(tR[:, :n], src_i, nhi, tR[:, :n], ALU.mult, ALU.add)
                        nc.vector.scalar_tensor_tensor(tI[:, :n], src_i, hr, tgt_i, ALU.mult, ALU.add)
                        nc.vector.scalar_tensor_tensor(tI[:, :n], src_r, hi, tI[:, :n], ALU.mult, ALU.add)
                        nc.vector.tensor_copy(tgt_r, tR[:, :n])
                        nc.vector.tensor_copy(tgt_i, tI[:, :n])
                # pass B: seeded in-place rescan (u -> h)
                for blk in range(4):
                    lr, li, nli = col(0, blk), col(1, blk), col(2, blk)
                    for j in range(L):
                        if j == 0:
                            pR = gsl(g5r, blk, b, 0, C)
                            pI = gsl(g5i, blk, b, 0, C)
                        else:
                            pR = ucol(Ur, blk, b, j - 1)
                            pI = ucol(Ui, blk, b, j - 1)
                        cR = ucol(Ur, blk, b, j)
                        cI = ucol(Ui, blk, b, j)
                        nc.vector.scalar_tensor_tensor(cI, pR, li, cI, ALU.mult, ALU.add)
                        nc.vector.scalar_tensor_tensor(cI, pI, lr, cI, ALU.mult, ALU.add)
                        nc.vector.scalar_tensor_tensor(cR, pI, nli, cR, ALU.mult, ALU.add)
                        nc.vector.scalar_tensor_tensor(cR, pR, lr, cR, ALU.mult, ALU.add)

            def phase2(b):
                for mt in range(NMT):
                    t0 = mt * 512
                    ys = []
                    for ob in range(4):
                        p2 = ps2.tile([128, 512], F32, tag="pm1", name="p2")
                        for kt in range(4):
                            rhs = ur4[:, kt:kt + 1, b:b + 1, mt * 32:(mt + 1) * 32, :].squeeze()
                            nc.tensor.matmul(
                                p2[:], cr_t[:, kt * 512 + ob * 128:kt * 512 + ob * 128 + 128],
                                rhs, start=(kt == 0), stop=False)
                        for kt in range(4):
                            rhs = ui4[:, kt:kt + 1, b:b + 1, mt * 32:(mt + 1) * 32, :].squeeze()
                            nc.tensor.matmul(
                                p2[:], ci_t[:, kt * 512 + ob * 128:kt * 512 + ob * 128 + 128],
                                rhs, start=False, stop=(kt == 3))
                        y = yp.tile([128, 512], BF16, tag="y")
                        nc.scalar.activation(y[:], p2[:], ACTF.Identity, bias=col(5, ob), scale=1.0)
                        ys.append(y)
                    # stats
                    s1 = pst.tile([1, 512], F32, tag="pstat", name="s1")
                    s2 = pst.tile([1, 512], F32, tag="pstat", name="s2")
                    y2s = []
                    for ob in range(4):
                        y2 = yp.tile([128, 512], BF16, tag="y2")
                        nc.scalar.activation(y2[:], ys[ob][:], ACTF.Square)
                        y2s.append(y2)
                    for ob in range(4):
                        nc.tensor.matmul(s1[:], ones_t[:, 0:1], ys[ob][:],
                                         start=(ob == 0), stop=(ob == 3))
                    for ob in range(4):
                        nc.tensor.matmul(s2[:], ones_t[:, 0:1], y2s[ob][:],
                                         start=(ob == 0), stop=(ob == 3))
                    mean = abp.tile([1, 512], F32, tag="mean")
                    ms = abp.tile([1, 512], F32, tag="ms")
                    var = abp.tile([1, 512], F32, tag="var")
                    sd = abp.tile([1, 512], F32, tag="sd")
                    A1 = abp.tile([1, 512], F32, tag="A1")
                    B1 = abp.tile([1, 512], F32, tag="B1")
                    nc.scalar.activation(mean[:], s1[:], ACTF.Copy, scale=1.0 / H)
                    nc.vector.tensor_tensor(ms[:], mean[:], mean[:], ALU.mult)
                    nc.vector.scalar_tensor_tensor(var[:], s2[:], 1.0 / H, ms[:], ALU.mult, ALU.subtract)
                    nc.scalar.activation(sd[:], var[:], ACTF.Sqrt, bias=eps_t[0:1, :])
                    nc.vector.reciprocal(A1[:], sd[:])
                    nc.vector.scalar_tensor_tensor(B1[:], mean[:], -1.0, A1[:], ALU.mult, ALU.mult)
                    Ab_h = abp.tile([1, 512], BF16, tag="Abh")
                    Bb_h = abp.tile([1, 512], BF16, tag="Bbh")
                    nc.vector.tensor_copy(Ab_h[:], A1[:])
                    nc.vector.tensor_copy(Bb_h[:], B1[:])
                    pA = pst.tile([128, 512], F32, tag="pstat", name="pA")
                    pB = pst.tile([128, 512], F32, tag="pstat", name="pB")
                    nc.tensor.matmul(pA[:], ones_t[0:1, :], Ab_h[:], start=True, stop=True)
                    nc.tensor.matmul(pB[:], ones_t[0:1, :], Bb_h[:], start=True, stop=True)
                    Ab = abp.tile([128, 512], F32, tag="Ab")
                    Bb = abp.tile([128, 512], F32, tag="Bb")
                    nc.scalar.copy(Ab[:], pA[:])
                    nc.scalar.copy(Bb[:], pB[:])
                    # MLP
                    P1s = []
                    for nb in range(16):
                        p3 = ps3.tile([128, 512], F32, tag="p3")
                        for kt in range(4):
                            nc.tensor.matmul(
                                p3[:], w1_t[:, kt * 2048 + nb * 128:kt * 2048 + nb * 128 + 128],
                                ys[kt][:], start=(kt == 0), stop=(kt == 3))
                        P1 = p1p.tile([128, 512], BF16, tag="P1", name=f"P1_{nb}")
                        nc.scalar.copy(P1[:], p3[:])
                        P1s.append(P1)
                    for ob in range(4):
                        p4 = ps4.tile([128, 512], F32, tag="p4", name=f"p4_{ob}")
                        for nb in range(16):
                            nc.tensor.matmul(
                                p4[:], w2_t[:, nb * 512 + ob * 128:nb * 512 + ob * 128 + 128],
                                P1s[nb][:], start=(nb == 0), stop=(nb == 15))
                        f1 = fp.tile([128, 512], F32, tag="f1")
                        outf = fp.tile([128, 512], F32, tag="outf")
                        nc.vector.tensor_tensor(f1[:], p4[:], Ab[:], ALU.mult)
                        nc.vector.scalar_tensor_tensor(f1[:], Bb[:], col(6, ob), f1[:], ALU.mult, ALU.add)
                        nc.scalar.activation(outf[:], f1[:], ACTF.Identity, bias=col(7, ob), scale=1.0)
                        nc.sync.dma_start(out=out[b, ob * 128:(ob + 1) * 128, t0:t0 + 512], in_=outf[:])

            mm1(0)
            mm1(1)
            scan(0)
            scan(1)
            phase2(0)
            phase2(1)

    nc.compile()
    return nc


def _consts(nu_log, theta_log, gamma_log, br, bi, cr, ci, ln_scale, ln_bias,
            W1, b1, W2, b2):
    lam = np.exp(-np.exp(nu_log.astype(np.float64)) + 1j * np.exp(theta_log.astype(np.float64)))
    gamma = np.exp(gamma_log.astype(np.float64))
    cols = {}
    cols[0] = lam.real
    cols[1] = lam.imag
    cols[2] = -lam.imag
    cols[3] = br.astype(np.float64) * gamma
    cols[4] = bi.astype(np.float64) * gamma
    cols[5] = (cr - ci).astype(np.float64)
    W1s = W1.astype(np.float64) * ln_scale.astype(np.float64)[:, None]
    colsum = W1s.sum(0)
    cb = ln_bias.astype(np.float64) @ W1.astype(np.float64) + b1.astype(np.float64)
    cols[6] = colsum @ W2.astype(np.float64)
    cols[7] = cb @ W2.astype(np.float64) + b2.astype(np.float64)
    for k in range(7):
        p = lam ** (L * (1 << k))
        cols[8 + 3 * k] = p.real
        cols[9 + 3 * k] = p.imag
        cols[10 + 3 * k] = -p.imag
    cst = np.zeros((128, 4 * 29), np.float32)
    for c, v in cols.items():
        for blk in range(4):
            cst[:, c * 4 + blk] = v[blk * 128:(blk + 1) * 128].astype(np.float32)
    return cst, gamma, W1s


def kernel(x, nu_log, theta_log, gamma_log, Br, br, Bi, bi,
           Cr, cr, Ci, ci, ln_scale, ln_bias, W1, b1, W2, b2):
    if "nc" not in _CACHE:
        _CACHE["nc"] = _build()
    nc = _CACHE["nc"]
    cst, gamma, W1s = _consts(nu_log, theta_log, gamma_log, br, bi, cr, ci,
                              ln_scale, ln_bias, W1, b1, W2, b2)
    bf = ml_dtypes.bfloat16
    g32 = gamma.astype(np.float32)
    Brg = (Br * g32[None, :]).astype(bf)
    Big = (Bi * g32[None, :]).astype(bf)
    Crb = Cr.astype(bf)
    Cin = (-Ci).astype(bf)
    W1sb = W1s.astype(np.float32).astype(bf)
    W2b = W2.astype(bf)
    xt = np.ascontiguousarray(x.transpose(0, 2, 1)).astype(bf)  # [B, H, T]
    in_maps = []
    for i in range(NCORES):
        in_maps.append(dict(x_t=xt[2 * i:2 * i + 2], Brg=Brg, Big=Big,
                            Crt=Crb, Cin=Cin, W1s=W1sb, W2t=W2b, cst=cst))
    res = run_bass_kernel_spmd(nc, in_maps, core_ids=list(range(NCORES)))
    out = np.empty((B, T, O), np.float32)
    for i in range(NCORES):
        o = res.results[i]["out_t"]  # [NB, O, T]
        out[2 * i:2 * i + 2] = o.transpose(0, 2, 1)
    return out



# revision 3
# speedup vs baseline: 2.4122x; 2.4122x over previous
import sys
sys.path.insert(0, '/opt/trn_rl_repo')
import numpy as np
import ml_dtypes
import concourse.bacc as bacc
import concourse.mybir as mybir
import concourse.tile as tile
from concourse.bass_utils import run_bass_kernel_spmd

F32 = mybir.dt.float32
BF16 = mybir.dt.bfloat16
ALU = mybir.AluOpType
ACTF = mybir.ActivationFunctionType

B, T, H, O = 16, 2048, 512, 512
NB = 2            # batch rows per core
NCORES = 8
NMT = T // 512    # 512-token tiles per row
SCH = 1024        # scan chunk length
LN_EPS = 1e-6

_CACHE = {}

# cst column layout: 5 consts x 4 blocks
C_BRG, C_BIG, C_CRCI, C_W12COL, C_OUTB = range(5)


def _build():
    nc = bacc.Bacc(None, target_bir_lowering=False)
    xin = nc.declare_dram_parameter("x_t", [NB, H, T], BF16, False)
    Brg = nc.declare_dram_parameter("Brg", [H, H], BF16, False)
    Big = nc.declare_dram_parameter("Big", [H, H], BF16, False)
    Crt = nc.declare_dram_parameter("Crt", [H, H], BF16, False)
    Cin = nc.declare_dram_parameter("Cin", [H, H], BF16, False)
    W12 = nc.declare_dram_parameter("W12", [H, H], BF16, False)
    TABS = nc.declare_dram_parameter("tabs", [8 * 128, T], BF16, False)
    RHO = nc.declare_dram_parameter("rho", [4 * 128, SCH], F32, False)
    CST = nc.declare_dram_parameter("cst", [128, 4 * 5], F32, False)
    out = nc.declare_dram_parameter("out_t", [NB, O, T], F32, True)

    with tile.TileContext(nc) as tc:
        with tc.tile_pool(name="wpool", bufs=1) as wp, \
             tc.tile_pool(name="upool", bufs=1) as up, \
             tc.tile_pool(name="tmpp", bufs=1) as tp, \
             tc.tile_pool(name="xp", bufs=2) as xp, \
             tc.tile_pool(name="yp", bufs=4) as yp, \
             tc.tile_pool(name="stp", bufs=2) as stp, \
             tc.tile_pool(name="ofp", bufs=4) as ofp, \
             tc.tile_pool(name="ps_mm1", bufs=2, space="PSUM") as ps1, \
             tc.tile_pool(name="ps_y", bufs=2, space="PSUM") as psy, \
             tc.tile_pool(name="ps_st", bufs=2, space="PSUM") as pst, \
             tc.tile_pool(name="ps_p4", bufs=2, space="PSUM") as ps4:

            # ---- resident weights / tables ----
            brg_t = wp.tile([128, 4 * 512], BF16, tag="brg")
            big_t = wp.tile([128, 4 * 512], BF16, tag="big")
            cr_t = wp.tile([128, 4 * 512], BF16, tag="cr")
            ci_t = wp.tile([128, 4 * 512], BF16, tag="ci")
            w12_t = wp.tile([128, 4 * 512], BF16, tag="w12")
            tab_t = wp.tile([128, 8 * T], BF16, tag="tabs")
            rho_t = wp.tile([128, 4 * SCH], F32, tag="rho")
            cst_t = wp.tile([128, 4 * 5], F32, tag="cst")
            ones_t = wp.tile([128, 128], BF16, tag="ones")
            eps_t = wp.tile([128, 1], F32, tag="eps")
            for (dst, src) in ((brg_t, Brg), (big_t, Big), (cr_t, Crt),
                               (ci_t, Cin), (w12_t, W12)):
                nc.sync.dma_start(
                    out=dst[:].rearrange("p (k n) -> p k n", k=4),
                    in_=src[:].rearrange("(k p) n -> p k n", p=128))
            nc.sync.dma_start(
                out=tab_t[:].rearrange("p (g t) -> p g t", g=8),
                in_=TABS[:].rearrange("(g p) t -> p g t", p=128))
            nc.sync.dma_start(
                out=rho_t[:].rearrange("p (g t) -> p g t", g=4),
                in_=RHO[:].rearrange("(g p) t -> p g t", p=128))
            nc.sync.dma_start(out=cst_t[:], in_=CST[:])
            nc.vector.memset(ones_t[:], 1.0)
            nc.vector.memset(eps_t[:], LN_EPS)

            def col(c, blk):
                return cst_t[:, c * 4 + blk:c * 4 + blk + 1]

            def ctab(bk):
                return tab_t[:, (2 * bk) * T:(2 * bk + 1) * T]

            def stab(bk):
                return tab_t[:, (2 * bk + 1) * T:(2 * bk + 2) * T]

            # u/h storage: per (b, Bk): R and I planes, token-contiguous
            U = up.tile([128, NB * 4 * 2 * T], BF16, tag="u")
            uv = U[:].rearrange("p (b k c t) -> p b k c t", b=NB, k=4, c=2)

            def uplane(b, bk, c):
                return uv[:, b:b + 1, bk:bk + 1, c:c + 1, :].squeeze()

            tmp1 = tp.tile([128, T], BF16, tag="tmp1")
            tmp2 = tp.tile([128, T], BF16, tag="tmp2")
            tmp3 = tp.tile([128, T], BF16, tag="tmp3")

            def mm1(b):
                for mt in range(NMT):
                    t0 = mt * 512
                    xt = xp.tile([128, 4 * 512], BF16, tag="xt")
                    nc.sync.dma_start(
                        out=xt[:].rearrange("p (k t) -> p k t", k=4),
                        in_=xin[b, :, t0:t0 + 512].rearrange("(k p) t -> p k t", p=128))
                    for (wt, c, bcol) in ((brg_t, 0, C_BRG), (big_t, 1, C_BIG)):
                        for ob in range(4):
                            pm = ps1.tile([128, 512], F32, tag="pm1")
                            for kt in range(4):
                                nc.tensor.matmul(
                                    pm[:], wt[:, kt * 512 + ob * 128:kt * 512 + ob * 128 + 128],
                                    xt[:, kt * 512:(kt + 1) * 512],
                                    start=(kt == 0), stop=(kt == 3))
                            nc.scalar.activation(
                                uplane(b, ob, c)[:, t0:t0 + 512], pm[:],
                                ACTF.Identity, bias=col(bcol, ob), scale=1.0)

            def unit(b, bk):
                uR = uplane(b, bk, 0)
                uI = uplane(b, bk, 1)
                c_, s_ = ctab(bk), stab(bk)
                # rotate: v = e^{-i theta s} * u   (in place)
                nc.vector.tensor_tensor(tmp1[:], c_, uR, ALU.mult)
                nc.vector.tensor_tensor(tmp2[:], s_, uR, ALU.mult)
                nc.vector.tensor_tensor(tmp3[:], s_, uI, ALU.mult)
                nc.vector.tensor_tensor(uR, tmp1[:], tmp3[:], ALU.add)
                nc.vector.tensor_tensor(tmp1[:], c_, uI, ALU.mult)
                nc.vector.tensor_tensor(uI, tmp1[:], tmp2[:], ALU.subtract)
                # scan: w = cumsum with decay rho (in place), chained chunks
                rho = rho_t[:, bk * SCH:(bk + 1) * SCH]
                for c in (0, 1):
                    pl = uplane(b, bk, c)
                    for k in range(T // SCH):
                        ini = 0.0 if k == 0 else pl[:, k * SCH - 1:k * SCH]
                        nc.vector.tensor_tensor_scan(
                            pl[:, k * SCH:(k + 1) * SCH], rho,
                            pl[:, k * SCH:(k + 1) * SCH], ini,
                            ALU.mult, ALU.add)
                # unrotate: h = e^{i theta t} * w   (in place)
                nc.vector.tensor_tensor(tmp1[:], c_, uR, ALU.mult)
                nc.vector.tensor_tensor(tmp2[:], s_, uR, ALU.mult)
                nc.vector.tensor_tensor(tmp3[:], s_, uI, ALU.mult)
                nc.vector.tensor_tensor(uR, tmp1[:], tmp3[:], ALU.subtract)
                nc.vector.tensor_tensor(tmp1[:], c_, uI, ALU.mult)
                nc.vector.tensor_tensor(uI, tmp2[:], tmp1[:], ALU.add)

            def phase2(b):
                for mt in range(NMT):
                    t0 = mt * 512
                    ys = []
                    y2s = []
                    for ob in range(4):
                        p2 = psy.tile([128, 512], F32, tag="py")
                        for bk in range(4):
                            nc.tensor.matmul(
                                p2[:], cr_t[:, bk * 512 + ob * 128:bk * 512 + ob * 128 + 128],
                                uplane(b, bk, 0)[:, t0:t0 + 512],
                                start=(bk == 0), stop=False)
                        for bk in range(4):
                            nc.tensor.matmul(
                                p2[:], ci_t[:, bk * 512 + ob * 128:bk * 512 + ob * 128 + 128],
                                uplane(b, bk, 1)[:, t0:t0 + 512],
                                start=False, stop=(bk == 3))
                        y = yp.tile([128, 512], BF16, tag="y", name=f"y{ob}")
                        y2 = yp.tile([128, 512], BF16, tag="y2", name=f"y2_{ob}")
                        nc.scalar.activation(y[:], p2[:], ACTF.Identity,
                                             bias=col(C_CRCI, ob), scale=1.0)
                        nc.scalar.activation(y2[:], p2[:], ACTF.Square,
                                             bias=col(C_CRCI, ob), scale=1.0)
                        ys.append(y)
                        y2s.append(y2)
                    # broadcast stats: s1b/s2b [128,512] = column sums
                    s1 = pst.tile([128, 512], F32, tag="pstat", name="s1")
                    s2 = pst.tile([128, 512], F32, tag="pstat", name="s2")
                    for ob in range(4):
                        nc.tensor.matmul(s1[:], ones_t[:], ys[ob][:],
                                         start=(ob == 0), stop=(ob == 3))
                    for ob in range(4):
                        nc.tensor.matmul(s2[:], ones_t[:], y2s[ob][:],
                                         start=(ob == 0), stop=(ob == 3))
                    mean = stp.tile([128, 512], F32, tag="mean")
                    ms = stp.tile([128, 512], F32, tag="ms")
                    var = stp.tile([128, 512], F32, tag="var")
                    sd = stp.tile([128, 512], F32, tag="sd")
                    A1 = stp.tile([128, 512], F32, tag="A1")
                    B1 = stp.tile([128, 512], F32, tag="B1")
                    nc.scalar.activation(mean[:], s1[:], ACTF.Copy, scale=1.0 / H)
                    nc.scalar.activation(ms[:], mean[:], ACTF.Square)
                    nc.vector.scalar_tensor_tensor(var[:], s2[:], 1.0 / H, ms[:],
                                                   ALU.mult, ALU.subtract)
                    nc.scalar.activation(sd[:], var[:], ACTF.Sqrt, bias=eps_t[:, :])
                    nc.vector.reciprocal_approx_fast(A1[:], sd[:])
                    nc.vector.scalar_tensor_tensor(B1[:], mean[:], -1.0, A1[:],
                                                   ALU.mult, ALU.mult)
                    # MLP collapsed: p4 = y @ W12
                    for ob in range(4):
                        p4 = ps4.tile([128, 512], F32, tag="p4")
                        for kt in range(4):
                            nc.tensor.matmul(
                                p4[:], w12_t[:, kt * 512 + ob * 128:kt * 512 + ob * 128 + 128],
                                ys[kt][:], start=(kt == 0), stop=(kt == 3))
                        f1 = ofp.tile([128, 512], F32, tag="f1")
                        outf = ofp.tile([128, 512], F32, tag="outf")
                        nc.vector.tensor_tensor(f1[:], p4[:], A1[:], ALU.mult)
                        nc.vector.scalar_tensor_tensor(f1[:], B1[:], col(C_W12COL, ob),
                                                       f1[:], ALU.mult, ALU.add)
                        nc.scalar.activation(outf[:], f1[:], ACTF.Identity,
                                             bias=col(C_OUTB, ob), scale=1.0)
                        nc.sync.dma_start(out=out[b, ob * 128:(ob + 1) * 128, t0:t0 + 512],
                                          in_=outf[:])

            mm1(0)
            for bk in range(4):
                unit(0, bk)
            mm1(1)
            for bk in range(4):
                unit(1, bk)
            phase2(0)
            phase2(1)

    nc.compile()
    return nc


def _consts(nu_log, theta_log, gamma_log, br, bi, cr, ci, ln_scale, ln_bias,
            W1, b1, W2, b2):
    nu = np.exp(nu_log.astype(np.float64))
    theta = np.exp(theta_log.astype(np.float64))
    rho = np.exp(-nu)                       # |lambda|
    gamma = np.exp(gamma_log.astype(np.float64))
    W1s = W1.astype(np.float64) * ln_scale.astype(np.float64)[:, None]
    W12 = W1s @ W2.astype(np.float64)
    cols = {}
    cols[C_BRG] = br.astype(np.float64) * gamma
    cols[C_BIG] = bi.astype(np.float64) * gamma
    cols[C_CRCI] = (cr - ci).astype(np.float64)
    cols[C_W12COL] = W12.sum(0)
    cols[C_OUTB] = (ln_bias.astype(np.float64) @ W1.astype(np.float64)
                    + b1.astype(np.float64)) @ W2.astype(np.float64) + b2.astype(np.float64)
    cst = np.zeros((128, 4 * 5), np.float32)
    for c, v in cols.items():
        for blk in range(4):
            cst[:, c * 4 + blk] = v[blk * 128:(blk + 1) * 128].astype(np.float32)
    # twiddle tables: per Bk block, cos/sin(theta_h * t), [8*128, T]
    t_idx = np.arange(T, dtype=np.float64)
    ang = theta[:, None] * t_idx[None, :]          # [H, T]
    bf = ml_dtypes.bfloat16
    tabs = np.zeros((8 * 128, T), bf)
    for blk in range(4):
        hs = slice(blk * 128, (blk + 1) * 128)
        tabs[2 * blk * 128:(2 * blk + 1) * 128] = np.cos(ang[hs]).astype(bf)
        tabs[(2 * blk + 1) * 128:(2 * blk + 2) * 128] = np.sin(ang[hs]).astype(bf)
    rho_tab = np.repeat(rho.astype(np.float32)[:, None], SCH, axis=1)  # [512, SCH]
    return cst, tabs, rho_tab, gamma, W12


def kernel(x, nu_log, theta_log, gamma_log, Br, br, Bi, bi,
           Cr, cr, Ci, ci, ln_scale, ln_bias, W1, b1, W2, b2):
    if "nc" not in _CACHE:
        _CACHE["nc"] = _build()
    nc = _CACHE["nc"]
    cst, tabs, rho_tab, gamma, W12 = _consts(
        nu_log, theta_log, gamma_log, br, bi, cr, ci,
        ln_scale, ln_bias, W1, b1, W2, b2)
    bf = ml_dtypes.bfloat16
    g32 = gamma.astype(np.float32)
    Brg = (Br * g32[None, :]).astype(bf)
    Big = (Bi * g32[None, :]).astype(bf)
    Crb = Cr.astype(bf)
    Cinb = (-Ci).astype(bf)
    W12b = W12.astype(np.float32).astype(bf)
    xt = np.ascontiguousarray(x.transpose(0, 2, 1)).astype(bf)  # [B, H, T]
    in_maps = []
    for i in range(NCORES):
        in_maps.append(dict(x_t=xt[2 * i:2 * i + 2], Brg=Brg, Big=Big,
                            Crt=Crb, Cin=Cinb, W12=W12b, tabs=tabs,
                            rho=rho_tab, cst=cst))
    res = run_bass_kernel_spmd(nc, in_maps, core_ids=list(range(NCORES)))
    out = np.empty((B, T, O), np.float32)
    for i in range(NCORES):
        o = res.results[i]["out_t"]  # [NB, O, T]
        out[2 * i:2 * i + 2] = o.transpose(0, 2, 1)
    return out


# revision 7
# speedup vs baseline: 2.9204x; 1.2107x over previous
import sys
sys.path.insert(0, '/opt/trn_rl_repo')
import numpy as np
import ml_dtypes
import concourse.bacc as bacc
import concourse.mybir as mybir
import concourse.tile as tile
from concourse.bass_utils import run_bass_kernel_spmd

F32 = mybir.dt.float32
BF16 = mybir.dt.bfloat16
ALU = mybir.AluOpType
ACTF = mybir.ActivationFunctionType

B, T, H, O = 16, 2048, 512, 512
NB = 2            # batch rows per core
NCORES = 8
NMT = T // 512    # 512-token tiles per row
SCH = 1024        # scan chunk length (= half of T)
LN_EPS = 1e-6

_CACHE = {}

# cst column layout: 3 consts x 4 blocks
C_BRG, C_BIG, C_CRCI = range(3)


def _build():
    nc = bacc.Bacc(None, target_bir_lowering=False)
    xin = nc.declare_dram_parameter("x_t", [NB, H, T], BF16, False)
    Brg = nc.declare_dram_parameter("Brg", [H, H], BF16, False)
    Big = nc.declare_dram_parameter("Big", [H, H], BF16, False)
    Crt = nc.declare_dram_parameter("Crt", [H, H], BF16, False)
    Cin = nc.declare_dram_parameter("Cin", [H, H], BF16, False)
    W12 = nc.declare_dram_parameter("W12", [H, H], BF16, False)
    TABS = nc.declare_dram_parameter("tabs", [8 * 128, T], BF16, False)
    RHO = nc.declare_dram_parameter("rho", [4 * 128, SCH], F32, False)
    CST = nc.declare_dram_parameter("cst", [128, 4 * 3], F32, False)
    out = nc.declare_dram_parameter("out_t", [NB, T, O], F32, True)

    with tile.TileContext(nc) as tc:
        with tc.tile_pool(name="wpool", bufs=1) as wp, \
             tc.tile_pool(name="upool", bufs=1) as up, \
             tc.tile_pool(name="tmpp", bufs=1) as tp, \
             tc.tile_pool(name="xp", bufs=5) as xp, \
             tc.tile_pool(name="yp", bufs=4) as yp, \
             tc.tile_pool(name="stp", bufs=2) as stp, \
             tc.tile_pool(name="ofp", bufs=4) as ofp, \
             tc.tile_pool(name="ps_mm1", bufs=2, space="PSUM") as ps1, \
             tc.tile_pool(name="ps_y", bufs=2, space="PSUM") as psy, \
             tc.tile_pool(name="ps_st", bufs=1, space="PSUM") as pst, \
             tc.tile_pool(name="ps_p4", bufs=2, space="PSUM") as ps4:

            # ---- early weights (mm1 path) ----
            brg_t = wp.tile([128, 4 * 512], BF16, tag="brg")
            big_t = wp.tile([128, 4 * 512], BF16, tag="big")
            cst_t = wp.tile([128, 4 * 3], F32, tag="cst")
            for (dst, src) in ((brg_t, Brg), (big_t, Big)):
                nc.sync.dma_start(
                    out=dst[:].rearrange("p (k n) -> p k n", k=4),
                    in_=src[:].rearrange("(k p) n -> p k n", p=128))
            nc.sync.dma_start(out=cst_t[:], in_=CST[:])

            cr_t = wp.tile([128, 4 * 512], BF16, tag="cr")
            ci_t = wp.tile([128, 4 * 512], BF16, tag="ci")
            w12_t = wp.tile([128, 4 * 512], BF16, tag="w12")
            tab_t = wp.tile([128, 8 * T], BF16, tag="tabs")
            rho_t = wp.tile([128, 4 * SCH], F32, tag="rho")
            ones_t = wp.tile([128, 128], BF16, tag="ones")
            ones32 = wp.tile([1, 1], F32, tag="ones32")
            eps_t = wp.tile([128, 1], F32, tag="eps")
            nc.vector.memset(ones_t[:], 1.0)
            nc.vector.memset(ones32[:], 1.0)
            nc.vector.memset(eps_t[:], LN_EPS)

            def col(c, blk):
                return cst_t[:, c * 4 + blk:c * 4 + blk + 1]

            def ctab(bk):
                return tab_t[:, (2 * bk) * T:(2 * bk + 1) * T]

            def stab(bk):
                return tab_t[:, (2 * bk + 1) * T:(2 * bk + 2) * T]

            # u/h storage: per (b, Bk): R and I planes, token-contiguous
            U = up.tile([128, NB * 4 * 2 * T], BF16, tag="u")
            uv = U[:].rearrange("p (b k c t) -> p b k c t", b=NB, k=4, c=2)

            def uplane(b, bk, c):
                return uv[:, b:b + 1, bk:bk + 1, c:c + 1, :].squeeze()

            tmp1 = tp.tile([128, SCH], BF16, tag="tmp1")
            tmp2 = tp.tile([128, SCH], BF16, tag="tmp2")
            tmp3 = tp.tile([128, SCH], BF16, tag="tmp3")
            carry = tp.tile([128, 2], BF16, tag="carry")

            def mm1(b):
                xts = []
                for mt in range(NMT):
                    t0 = mt * 512
                    xt = xp.tile([128, 4 * 512], BF16, tag="xt")
                    nc.sync.dma_start(
                        out=xt[:].rearrange("p (k t) -> p k t", k=4),
                        in_=xin[b, :, t0:t0 + 512].rearrange("(k p) t -> p k t", p=128))
                    xts.append(xt)
                for ob in range(4):
                    for mt in range(NMT):
                        t0 = mt * 512
                        for (wt, c, bcol) in ((brg_t, 0, C_BRG), (big_t, 1, C_BIG)):
                            pm = ps1.tile([128, 512], F32, tag="pm1")
                            for kt in range(4):
                                nc.tensor.matmul(
                                    pm[:], wt[:, kt * 512 + ob * 128:kt * 512 + ob * 128 + 128],
                                    xts[mt][:, kt * 512:(kt + 1) * 512],
                                    start=(kt == 0), stop=(kt == 3))
                            nc.scalar.activation(
                                uplane(b, ob, c)[:, t0:t0 + 512], pm[:],
                                ACTF.Identity, bias=col(bcol, ob), scale=1.0)

            def unit(b, bk):
                uR = uplane(b, bk, 0)
                uI = uplane(b, bk, 1)
                rho = rho_t[:, bk * SCH:(bk + 1) * SCH]
                for hf in range(T // SCH):
                    s0 = hf * SCH
                    sl = slice(s0, s0 + SCH)
                    c_, s_ = ctab(bk)[:, sl], stab(bk)[:, sl]
                    # rotate: v = e^{-i theta s} * u   (in place)
                    nc.vector.tensor_tensor(tmp1[:], c_, uR[:, sl], ALU.mult)
                    nc.vector.tensor_tensor(tmp2[:], s_, uR[:, sl], ALU.mult)
                    nc.vector.tensor_tensor(tmp3[:], s_, uI[:, sl], ALU.mult)
                    nc.vector.tensor_tensor(uR[:, sl], tmp1[:], tmp3[:], ALU.add)
                    nc.vector.tensor_tensor(tmp1[:], c_, uI[:, sl], ALU.mult)
                    nc.vector.tensor_tensor(uI[:, sl], tmp1[:], tmp2[:], ALU.subtract)
                    # scan: w = cumsum with decay rho (in place)
                    for ci, pl in ((0, uR), (1, uI)):
                        ini = 0.0 if hf == 0 else carry[:, ci:ci + 1]
                        nc.vector.tensor_tensor_scan(
                            pl[:, sl], rho, pl[:, sl], ini, ALU.mult, ALU.add)
                    if hf == 0:
                        # save chunk-boundary state before in-place unrotate
                        nc.vector.tensor_copy(carry[:, 0:1], uR[:, s0 + SCH - 1:s0 + SCH])
                        nc.vector.tensor_copy(carry[:, 1:2], uI[:, s0 + SCH - 1:s0 + SCH])
                    # unrotate: h = e^{i theta t} * w   (in place)
                    nc.vector.tensor_tensor(tmp1[:], c_, uR[:, sl], ALU.mult)
                    nc.vector.tensor_tensor(tmp2[:], s_, uR[:, sl], ALU.mult)
                    nc.vector.tensor_tensor(tmp3[:], s_, uI[:, sl], ALU.mult)
                    nc.vector.tensor_tensor(uR[:, sl], tmp1[:], tmp3[:], ALU.subtract)
                    nc.vector.tensor_tensor(tmp1[:], c_, uI[:, sl], ALU.mult)
                    nc.vector.tensor_tensor(uI[:, sl], tmp2[:], tmp1[:], ALU.add)

            def phase2_mt(b, mt):
                t0 = mt * 512
                ys = []
                y2s = []
                for ob in range(4):
                    p2 = psy.tile([128, 512], F32, tag="py")
                    for bk in range(4):
                        nc.tensor.matmul(
                            p2[:], cr_t[:, bk * 512 + ob * 128:bk * 512 + ob * 128 + 128],
                            uplane(b, bk, 0)[:, t0:t0 + 512],
                            start=(bk == 0), stop=False)
                    for bk in range(4):
                        nc.tensor.matmul(
                            p2[:], ci_t[:, bk * 512 + ob * 128:bk * 512 + ob * 128 + 128],
                            uplane(b, bk, 1)[:, t0:t0 + 512],
                            start=False, stop=(bk == 3))
                    y = yp.tile([128, 512], BF16, tag="y", name=f"y{ob}")
                    y2 = yp.tile([128, 512], BF16, tag="y2", name=f"y2_{ob}")
                    nc.scalar.activation(y[:], p2[:], ACTF.Identity,
                                         bias=col(C_CRCI, ob), scale=1.0)
                    nc.scalar.activation(y2[:], p2[:], ACTF.Square,
                                         bias=col(C_CRCI, ob), scale=1.0)
                    ys.append(y)
                    y2s.append(y2)
                # per-token stats [1, 512]
                s1 = pst.tile([1, 512], F32, tag="s1", name="s1")
                s2 = pst.tile([1, 512], F32, tag="s2", name="s2")
                for ob in range(4):
                    nc.tensor.matmul(s1[:], ones_t[:, 0:1], ys[ob][:],
                                     start=(ob == 0), stop=(ob == 3))
                for ob in range(4):
                    nc.tensor.matmul(s2[:], ones_t[:, 0:1], y2s[ob][:],
                                     start=(ob == 0), stop=(ob == 3))
                mean = stp.tile([1, 512], F32, tag="mean")
                ms = stp.tile([1, 512], F32, tag="ms")
                var = stp.tile([1, 512], F32, tag="var")
                sd = stp.tile([1, 512], F32, tag="sd")
                A1 = stp.tile([1, 512], F32, tag="A1")
                A1t = stp.tile([128, 4], F32, tag="A1t")
                nc.scalar.activation(mean[:], s1[:], ACTF.Copy, scale=1.0 / H)
                nc.scalar.activation(ms[:], mean[:], ACTF.Square)
                nc.vector.scalar_tensor_tensor(var[:], s2[:], 1.0 / H, ms[:],
                                               ALU.mult, ALU.subtract)
                nc.scalar.activation(sd[:], var[:], ACTF.Sqrt, bias=eps_t[0:1, :])
                nc.vector.reciprocal_approx_fast(A1[:], sd[:])
                # transpose A1 [1,512] -> [128,4] via PE: col tb = A1-slice^T @ [1]
                pa = ps4.tile([128, 512], F32, tag="p4", name="pa1t")
                for tb in range(4):
                    nc.tensor.matmul(pa[:, tb:tb + 1],
                                     A1[:, tb * 128:(tb + 1) * 128],
                                     ones32[:], start=True, stop=True)
                nc.scalar.activation(A1t[:], pa[:, 0:4], ACTF.Copy)
                # MLP collapsed + LN fold: p4t[t, o] = sum_k y[k,t] * W12c[k,o]
                for tb in range(4):
                    p4 = ps4.tile([128, 512], F32, tag="p4")
                    for kt in range(4):
                        nc.tensor.matmul(
                            p4[:], ys[kt][:, tb * 128:(tb + 1) * 128],
                            w12_t[:, kt * 512:(kt + 1) * 512],
                            start=(kt == 0), stop=(kt == 3))
                    outf = ofp.tile([128, 512], F32, tag="outf")
                    nc.scalar.activation(outf[:], p4[:], ACTF.Copy,
                                         scale=A1t[:, tb:tb + 1])
                    nc.sync.dma_start(
                        out=out[b, t0 + tb * 128:t0 + (tb + 1) * 128, :],
                        in_=outf[:])

            # ---- emission order (pipelining) ----
            mm1(0)
            # bulk weights/tables after mm1(0) DMAs so mm1 starts immediately
            nc.sync.dma_start(
                out=tab_t[:].rearrange("p (g t) -> p g t", g=8),
                in_=TABS[:].rearrange("(g p) t -> p g t", p=128))
            nc.sync.dma_start(
                out=rho_t[:].rearrange("p (g t) -> p g t", g=4),
                in_=RHO[:].rearrange("(g p) t -> p g t", p=128))
            for bk in range(4):
                unit(0, bk)
            for (dst, src) in ((cr_t, Crt), (ci_t, Cin), (w12_t, W12)):
                nc.sync.dma_start(
                    out=dst[:].rearrange("p (k n) -> p k n", k=4),
                    in_=src[:].rearrange("(k p) n -> p k n", p=128))
            mm1(1)
            for k in range(4):
                unit(1, k)
                phase2_mt(0, k)
            for mt in range(NMT):
                phase2_mt(1, mt)

    nc.compile()
    return nc


def _consts(nu_log, theta_log, gamma_log, br, bi, cr, ci, ln_scale, ln_bias,
            W1, b1, W2, b2):
    nu = np.exp(nu_log.astype(np.float64))
    theta = np.exp(theta_log.astype(np.float64))
    rho = np.exp(-nu)                       # |lambda|
    gamma = np.exp(gamma_log.astype(np.float64))
    W1s = W1.astype(np.float64) * ln_scale.astype(np.float64)[:, None]
    W12 = W1s @ W2.astype(np.float64)
    col6 = W12.sum(0)                        # ln_scale @ W1 @ W2
    # fold -mean*col6 into the weights: W12c = W12 - ones*col6/H
    W12c = W12 - col6[None, :] / H
    cols7 = ((ln_bias.astype(np.float64) @ W1.astype(np.float64)
              + b1.astype(np.float64)) @ W2.astype(np.float64)
             + b2.astype(np.float64)).astype(np.float32)
    cols = {}
    cols[C_BRG] = br.astype(np.float64) * gamma
    cols[C_BIG] = bi.astype(np.float64) * gamma
    cols[C_CRCI] = (cr - ci).astype(np.float64)
    cst = np.zeros((128, 4 * 3), np.float32)
    for c, v in cols.items():
        for blk in range(4):
            cst[:, c * 4 + blk] = v[blk * 128:(blk + 1) * 128].astype(np.float32)
    # twiddle tables: per Bk block, cos/sin(theta_h * t), [8*128, T]
    t_idx = np.arange(T, dtype=np.float64)
    ang = theta[:, None] * t_idx[None, :]          # [H, T]
    bf = ml_dtypes.bfloat16
    tabs = np.zeros((8 * 128, T), bf)
    for blk in range(4):
        hs = slice(blk * 128, (blk + 1) * 128)
        tabs[2 * blk * 128:(2 * blk + 1) * 128] = np.cos(ang[hs]).astype(bf)
        tabs[(2 * blk + 1) * 128:(2 * blk + 2) * 128] = np.sin(ang[hs]).astype(bf)
    rho_tab = np.repeat(rho.astype(np.float32)[:, None], SCH, axis=1)  # [512, SCH]
    return cst, tabs, rho_tab, gamma, W12c, cols7


def kernel(x, nu_log, theta_log, gamma_log, Br, br, Bi, bi,
           Cr, cr, Ci, ci, ln_scale, ln_bias, W1, b1, W2, b2):
    if "nc" not in _CACHE:
        _CACHE["nc"] = _build()
    nc = _CACHE["nc"]
    cst, tabs, rho_tab, gamma, W12c, cols7 = _consts(
        nu_log, theta_log, gamma_log, br, bi, cr, ci,
        ln_scale, ln_bias, W1, b1, W2, b2)
    bf = ml_dtypes.bfloat16
    g32 = gamma.astype(np.float32)
    Brg = (Br * g32[None, :]).astype(bf)
    Big = (Bi * g32[None, :]).astype(bf)
    Crb = Cr.astype(bf)
    Cinb = (-Ci).astype(bf)
    W12b = W12c.astype(np.float32).astype(bf)
    xt = np.ascontiguousarray(x.transpose(0, 2, 1)).astype(bf)  # [B, H, T]
    in_maps = []
    for i in range(NCORES):
        in_maps.append(dict(x_t=xt[2 * i:2 * i + 2], Brg=Brg, Big=Big,
                            Crt=Crb, Cin=Cinb, W12=W12b, tabs=tabs,
                            rho=rho_tab, cst=cst))
    res = run_bass_kernel_spmd(nc, in_maps, core_ids=list(range(NCORES)))
    out = np.empty((B, T, O), np.float32)
    for i in range(NCORES):
        out[2 * i:2 * i + 2] = res.results[i]["out_t"]  # [NB, T, O]
    if np.any(cols7):
        out += cols7[None, None, :]
    return out


# revision 11
# speedup vs baseline: 3.1118x; 1.0656x over previous
import sys
sys.path.insert(0, '/opt/trn_rl_repo')
import numpy as np
import ml_dtypes
import concourse.bacc as bacc
import concourse.mybir as mybir
import concourse.tile as tile
from concourse.bass_utils import run_bass_kernel_spmd

F32 = mybir.dt.float32
BF16 = mybir.dt.bfloat16
ALU = mybir.AluOpType
ACTF = mybir.ActivationFunctionType

B, T, H, O = 16, 2048, 512, 512
NB = 2            # batch rows per core
NCORES = 8
NMT = T // 512    # 512-token tiles per row
SCH = 1024        # scan chunk length (= half of T)
LN_EPS = 1e-6

_CACHE = {}

# cst column layout: 3 consts x 4 blocks
C_BRG, C_BIG, C_CRCI = range(3)


def _build():
    nc = bacc.Bacc(None, target_bir_lowering=False)
    xin = nc.declare_dram_parameter("x_t", [NB, H, T], BF16, False)
    Brg = nc.declare_dram_parameter("Brg", [H, H], BF16, False)
    Big = nc.declare_dram_parameter("Big", [H, H], BF16, False)
    Crt = nc.declare_dram_parameter("Crt", [H, H], BF16, False)
    Cin = nc.declare_dram_parameter("Cin", [H, H], BF16, False)
    W12 = nc.declare_dram_parameter("W12", [H, H], BF16, False)
    TABS = nc.declare_dram_parameter("tabs", [8 * 128, T], BF16, False)
    RHO = nc.declare_dram_parameter("rho", [4 * 128, SCH], F32, False)
    CST = nc.declare_dram_parameter("cst", [128, 4 * 3], F32, False)
    out = nc.declare_dram_parameter("out_t", [NB, T, O], F32, True)

    with tile.TileContext(nc) as tc:
        with tc.tile_pool(name="wpool", bufs=1) as wp, \
             tc.tile_pool(name="upool", bufs=1) as up, \
             tc.tile_pool(name="tmpp", bufs=1) as tp, \
             tc.tile_pool(name="xp", bufs=5) as xp, \
             tc.tile_pool(name="yp", bufs=4) as yp, \
             tc.tile_pool(name="stp", bufs=2) as stp, \
             tc.tile_pool(name="ofp", bufs=4) as ofp, \
             tc.tile_pool(name="ps_mm1", bufs=2, space="PSUM") as ps1, \
             tc.tile_pool(name="ps_y", bufs=2, space="PSUM") as psy, \
             tc.tile_pool(name="ps_st", bufs=1, space="PSUM") as pst, \
             tc.tile_pool(name="ps_p4", bufs=2, space="PSUM") as ps4:

            # ---- early weights (mm1 path) ----
            brg_t = wp.tile([128, 4 * 512], BF16, tag="brg")
            big_t = wp.tile([128, 4 * 512], BF16, tag="big")
            cst_t = wp.tile([128, 4 * 3], F32, tag="cst")
            for (dst, src) in ((brg_t, Brg), (big_t, Big)):
                nc.sync.dma_start(
                    out=dst[:].rearrange("p (k n) -> p k n", k=4),
                    in_=src[:].rearrange("(k p) n -> p k n", p=128))
            nc.sync.dma_start(out=cst_t[:], in_=CST[:])

            cr_t = wp.tile([128, 4 * 512], BF16, tag="cr")
            ci_t = wp.tile([128, 4 * 512], BF16, tag="ci")
            w12_t = wp.tile([128, 4 * 512], BF16, tag="w12")
            tab_t = wp.tile([128, 8 * T], BF16, tag="tabs")
            rho_t = wp.tile([128, 4 * SCH], F32, tag="rho")
            ones_t = wp.tile([128, 128], BF16, tag="ones")
            ones32 = wp.tile([1, 1], F32, tag="ones32")
            eps_t = wp.tile([128, 1], F32, tag="eps")
            nc.vector.memset(ones_t[:], 1.0)
            nc.vector.memset(ones32[:], 1.0)
            nc.vector.memset(eps_t[:], LN_EPS)

            def col(c, blk):
                return cst_t[:, c * 4 + blk:c * 4 + blk + 1]

            def ctab(bk):
                return tab_t[:, (2 * bk) * T:(2 * bk + 1) * T]

            def stab(bk):
                return tab_t[:, (2 * bk + 1) * T:(2 * bk + 2) * T]

            # u/h storage: per (b, Bk): R and I planes, token-contiguous
            U = up.tile([128, NB * 4 * 2 * T], BF16, tag="u")
            uv = U[:].rearrange("p (b k c t) -> p b k c t", b=NB, k=4, c=2)

            def uplane(b, bk, c):
                return uv[:, b:b + 1, bk:bk + 1, c:c + 1, :].squeeze()

            tmp1 = tp.tile([128, SCH], BF16, tag="tmp1")
            tmp2 = tp.tile([128, SCH], BF16, tag="tmp2")
            tmp3 = tp.tile([128, SCH], BF16, tag="tmp3")
            gmp1 = tp.tile([128, SCH], BF16, tag="gmp1")
            gmp2 = tp.tile([128, SCH], BF16, tag="gmp2")
            gmp3 = tp.tile([128, SCH], BF16, tag="gmp3")
            carry = tp.tile([128, 8], BF16, tag="carry")

            def mm1(b):
                xts = []
                for mt in range(NMT):
                    t0 = mt * 512
                    xt = xp.tile([128, 4 * 512], BF16, tag="xt")
                    nc.sync.dma_start(
                        out=xt[:].rearrange("p (k t) -> p k t", k=4),
                        in_=xin[b, :, t0:t0 + 512].rearrange("(k p) t -> p k t", p=128))
                    xts.append(xt)
                for ob in range(4):
                    for mt in range(NMT):
                        t0 = mt * 512
                        for (wt, c, bcol) in ((brg_t, 0, C_BRG), (big_t, 1, C_BIG)):
                            pm = ps1.tile([128, 512], F32, tag="pm1")
                            for kt in range(4):
                                nc.tensor.matmul(
                                    pm[:], wt[:, kt * 512 + ob * 128:kt * 512 + ob * 128 + 128],
                                    xts[mt][:, kt * 512:(kt + 1) * 512],
                                    start=(kt == 0), stop=(kt == 3))
                            nc.scalar.activation(
                                uplane(b, ob, c)[:, t0:t0 + 512], pm[:],
                                ACTF.Identity, bias=col(bcol, ob), scale=1.0)

            def unit_half(b, bk, hf, unrot=None):
                uR = uplane(b, bk, 0)
                uI = uplane(b, bk, 1)
                rho = rho_t[:, bk * SCH:(bk + 1) * SCH]
                s0 = hf * SCH
                sl = slice(s0, s0 + SCH)
                c_, s_ = ctab(bk)[:, sl], stab(bk)[:, sl]
                # rotate: v = e^{-i theta s} * u   (in place)
                nc.vector.tensor_tensor(tmp1[:], c_, uR[:, sl], ALU.mult)
                nc.vector.tensor_tensor(tmp2[:], s_, uR[:, sl], ALU.mult)
                nc.vector.tensor_tensor(tmp3[:], s_, uI[:, sl], ALU.mult)
                nc.vector.tensor_tensor(uR[:, sl], tmp1[:], tmp3[:], ALU.add)
                nc.vector.tensor_tensor(tmp1[:], c_, uI[:, sl], ALU.mult)
                nc.vector.tensor_tensor(uI[:, sl], tmp1[:], tmp2[:], ALU.subtract)
                # scan: w = cumsum with decay rho (in place)
                for ci, pl in ((0, uR), (1, uI)):
                    ini = 0.0 if hf == 0 else carry[:, bk * 2 + ci:bk * 2 + ci + 1]
                    nc.vector.tensor_tensor_scan(
                        pl[:, sl], rho, pl[:, sl], ini, ALU.mult, ALU.add)
                if hf == 0:
                    # save chunk-boundary state before in-place unrotate
                    nc.vector.tensor_copy(carry[:, bk * 2:bk * 2 + 1],
                                          uR[:, s0 + SCH - 1:s0 + SCH])
                    nc.vector.tensor_copy(carry[:, bk * 2 + 1:bk * 2 + 2],
                                          uI[:, s0 + SCH - 1:s0 + SCH])
                # unrotate: h = e^{i theta t} * w   (in place)
                if unrot is None:
                    eng, t1, t2, t3 = nc.vector, tmp1, tmp2, tmp3
                else:
                    eng, t1, t2, t3 = unrot, gmp1, gmp2, gmp3
                eng.tensor_tensor(t1[:], c_, uR[:, sl], ALU.mult)
                eng.tensor_tensor(t2[:], s_, uR[:, sl], ALU.mult)
                eng.tensor_tensor(t3[:], s_, uI[:, sl], ALU.mult)
                eng.tensor_tensor(uR[:, sl], t1[:], t3[:], ALU.subtract)
                eng.tensor_tensor(t1[:], c_, uI[:, sl], ALU.mult)
                eng.tensor_tensor(uI[:, sl], t2[:], t1[:], ALU.add)

            def phase2_mt(b, mt):
                t0 = mt * 512
                ys = []
                y2s = []
                for ob in range(4):
                    p2 = psy.tile([128, 512], F32, tag="py")
                    for bk in range(4):
                        nc.tensor.matmul(
                            p2[:], cr_t[:, bk * 512 + ob * 128:bk * 512 + ob * 128 + 128],
                            uplane(b, bk, 0)[:, t0:t0 + 512],
                            start=(bk == 0), stop=False)
                    for bk in range(4):
                        nc.tensor.matmul(
                            p2[:], ci_t[:, bk * 512 + ob * 128:bk * 512 + ob * 128 + 128],
                            uplane(b, bk, 1)[:, t0:t0 + 512],
                            start=False, stop=(bk == 3))
                    y = yp.tile([128, 512], BF16, tag="y", name=f"y{ob}")
                    y2 = yp.tile([128, 512], BF16, tag="y2", name=f"y2_{ob}")
                    nc.scalar.activation(y[:], p2[:], ACTF.Identity,
                                         bias=col(C_CRCI, ob), scale=1.0)
                    nc.scalar.activation(y2[:], p2[:], ACTF.Square,
                                         bias=col(C_CRCI, ob), scale=1.0)
                    ys.append(y)
                    y2s.append(y2)
                # per-token stats [1, 512]
                s1 = pst.tile([1, 512], F32, tag="s1", name="s1")
                s2 = pst.tile([1, 512], F32, tag="s2", name="s2")
                for ob in range(4):
                    nc.tensor.matmul(s1[:], ones_t[:, 0:1], ys[ob][:],
                                     start=(ob == 0), stop=(ob == 3))
                for ob in range(4):
                    nc.tensor.matmul(s2[:], ones_t[:, 0:1], y2s[ob][:],
                                     start=(ob == 0), stop=(ob == 3))
                mean = stp.tile([1, 512], F32, tag="mean")
                ms = stp.tile([1, 512], F32, tag="ms")
                var = stp.tile([1, 512], F32, tag="var")
                sd = stp.tile([1, 512], F32, tag="sd")
                A1 = stp.tile([1, 512], F32, tag="A1")
                A1t = stp.tile([128, 4], F32, tag="A1t")
                nc.scalar.activation(mean[:], s1[:], ACTF.Copy, scale=1.0 / H)
                nc.scalar.activation(ms[:], mean[:], ACTF.Square)
                nc.vector.scalar_tensor_tensor(var[:], s2[:], 1.0 / H, ms[:],
                                               ALU.mult, ALU.subtract)
                nc.scalar.activation(sd[:], var[:], ACTF.Sqrt, bias=eps_t[0:1, :])
                nc.vector.reciprocal_approx_fast(A1[:], sd[:])
                # transpose A1 [1,512] -> [128,4] via PE: col tb = A1-slice^T @ [1]
                pa = ps4.tile([128, 512], F32, tag="p4", name="pa1t")
                for tb in range(4):
                    nc.tensor.matmul(pa[:, tb:tb + 1],
                                     A1[:, tb * 128:(tb + 1) * 128],
                                     ones32[:], start=True, stop=True)
                nc.scalar.activation(A1t[:], pa[:, 0:4], ACTF.Copy)
                # MLP collapsed + LN fold: p4t[t, o] = sum_k y[k,t] * W12c[k,o]
                for tb in range(4):
                    p4 = ps4.tile([128, 512], F32, tag="p4")
                    for kt in range(4):
                        nc.tensor.matmul(
                            p4[:], ys[kt][:, tb * 128:(tb + 1) * 128],
                            w12_t[:, kt * 512:(kt + 1) * 512],
                            start=(kt == 0), stop=(kt == 3))
                    outf = ofp.tile([128, 512], F32, tag="outf")
                    nc.scalar.activation(outf[:], p4[:], ACTF.Copy,
                                         scale=A1t[:, tb:tb + 1])
                    nc.sync.dma_start(
                        out=out[b, t0 + tb * 128:t0 + (tb + 1) * 128, :],
                        in_=outf[:])

            # ---- emission order (pipelining) ----
            mm1(0)
            # bulk tables after mm1(0) DMAs so mm1 starts immediately
            nc.sync.dma_start(
                out=tab_t[:].rearrange("p (g t) -> p g t", g=8),
                in_=TABS[:].rearrange("(g p) t -> p g t", p=128))
            nc.sync.dma_start(
                out=rho_t[:].rearrange("p (g t) -> p g t", g=4),
                in_=RHO[:].rearrange("(g p) t -> p g t", p=128))
            for bk in range(4):
                unit_half(0, bk, 0)
            mm1(1)
            for (dst, src) in ((cr_t, Crt), (ci_t, Cin), (w12_t, W12)):
                nc.sync.dma_start(
                    out=dst[:].rearrange("p (k n) -> p k n", k=4),
                    in_=src[:].rearrange("(k p) n -> p k n", p=128))
            for bk in range(4):
                unit_half(0, bk, 1)
            # b1 halves interleaved with phase2(0); b1-h0 unrotate on gpsimd
            for k in range(4):
                unit_half(1, k, 0)
                phase2_mt(0, k)
            unit_half(1, 0, 1)
            phase2_mt(1, 0)
            unit_half(1, 1, 1)
            phase2_mt(1, 1)
            unit_half(1, 2, 1)
            unit_half(1, 3, 1)
            phase2_mt(1, 2)
            phase2_mt(1, 3)

    nc.compile()
    return nc


def _consts(nu_log, theta_log, gamma_log, br, bi, cr, ci, ln_scale, ln_bias,
            W1, b1, W2, b2):
    nu = np.exp(nu_log.astype(np.float64))
    theta = np.exp(theta_log.astype(np.float64))
    rho = np.exp(-nu)                       # |lambda|
    gamma = np.exp(gamma_log.astype(np.float64))
    W1s = W1.astype(np.float64) * ln_scale.astype(np.float64)[:, None]
    W12 = W1s @ W2.astype(np.float64)
    col6 = W12.sum(0)                        # ln_scale @ W1 @ W2
    # fold -mean*col6 into the weights: W12c = W12 - ones*col6/H
    W12c = W12 - col6[None, :] / H
    cols7 = ((ln_bias.astype(np.float64) @ W1.astype(np.float64)
              + b1.astype(np.float64)) @ W2.astype(np.float64)
             + b2.astype(np.float64)).astype(np.float32)
    cols = {}
    cols[C_BRG] = br.astype(np.float64) * gamma
    cols[C_BIG] = bi.astype(np.float64) * gamma
    cols[C_CRCI] = (cr - ci).astype(np.float64)
    cst = np.zeros((128, 4 * 3), np.float32)
    for c, v in cols.items():
        for blk in range(4):
            cst[:, c * 4 + blk] = v[blk * 128:(blk + 1) * 128].astype(np.float32)
    # twiddle tables: per Bk block, cos/sin(theta_h * t), [8*128, T]
    t_idx = np.arange(T, dtype=np.float64)
    ang = theta[:, None] * t_idx[None, :]          # [H, T]
    bf = ml_dtypes.bfloat16
    tabs = np.zeros((8 * 128, T), bf)
    for blk in range(4):
        hs = slice(blk * 128, (blk + 1) * 128)
        tabs[2 * blk * 128:(2 * blk + 1) * 128] = np.cos(ang[hs]).astype(bf)
        tabs[(2 * blk + 1) * 128:(2 * blk + 2) * 128] = np.sin(ang[hs]).astype(bf)
    rho_tab = np.repeat(rho.astype(np.float32)[:, None], SCH, axis=1)  # [512, SCH]
    return cst, tabs, rho_tab, gamma, W12c, cols7


def kernel(x, nu_log, theta_log, gamma_log, Br, br, Bi, bi,
           Cr, cr, Ci, ci, ln_scale, ln_bias, W1, b1, W2, b2):
    if "nc" not in _CACHE:
        _CACHE["nc"] = _build()
    nc = _CACHE["nc"]
    cst, tabs, rho_tab, gamma, W12c, cols7 = _consts(
        nu_log, theta_log, gamma_log, br, bi, cr, ci,
        ln_scale, ln_bias, W1, b1, W2, b2)
    bf = ml_dtypes.bfloat16
    g32 = gamma.astype(np.float32)
    Brg = (Br * g32[None, :]).astype(bf)
    Big = (Bi * g32[None, :]).astype(bf)
    Crb = Cr.astype(bf)
    Cinb = (-Ci).astype(bf)
    W12b = W12c.astype(np.float32).astype(bf)
    xt = np.ascontiguousarray(x.transpose(0, 2, 1)).astype(bf)  # [B, H, T]
    in_maps = []
    for i in range(NCORES):
        in_maps.append(dict(x_t=xt[2 * i:2 * i + 2], Brg=Brg, Big=Big,
                            Crt=Crb, Cin=Cinb, W12=W12b, tabs=tabs,
                            rho=rho_tab, cst=cst))
    res = run_bass_kernel_spmd(nc, in_maps, core_ids=list(range(NCORES)))
    out = np.empty((B, T, O), np.float32)
    for i in range(NCORES):
        out[2 * i:2 * i + 2] = res.results[i]["out_t"]  # [NB, T, O]
    if np.any(cols7):
        out += cols7[None, None, :]
    return out


# revision 14
# speedup vs baseline: 3.1174x; 1.0018x over previous
import sys
sys.path.insert(0, '/opt/trn_rl_repo')
import numpy as np
import ml_dtypes
import concourse.bacc as bacc
import concourse.mybir as mybir
import concourse.tile as tile
from concourse.bass_utils import run_bass_kernel_spmd

F32 = mybir.dt.float32
BF16 = mybir.dt.bfloat16
ALU = mybir.AluOpType
ACTF = mybir.ActivationFunctionType

B, T, H, O = 16, 2048, 512, 512
NB = 2            # batch rows per core
NCORES = 8
NMT = T // 512    # 512-token tiles per row
SCH = 1024        # scan chunk length (= half of T)
LN_EPS = 1e-6

_CACHE = {}

# cst column layout: 3 consts x 4 blocks
C_BRG, C_BIG, C_CRCI = range(3)


def _build():
    nc = bacc.Bacc(None, target_bir_lowering=False)
    xin = nc.declare_dram_parameter("x_t", [NB, H, T], BF16, False)
    Brg = nc.declare_dram_parameter("Brg", [H, H], BF16, False)
    Big = nc.declare_dram_parameter("Big", [H, H], BF16, False)
    Crt = nc.declare_dram_parameter("Crt", [H, H], BF16, False)
    Cin = nc.declare_dram_parameter("Cin", [H, H], BF16, False)
    W12 = nc.declare_dram_parameter("W12", [H, H], BF16, False)
    TABS = nc.declare_dram_parameter("tabs", [8 * 128, T], BF16, False)
    RHO = nc.declare_dram_parameter("rho", [4 * 128, SCH], F32, False)
    CST = nc.declare_dram_parameter("cst", [128, 4 * 3], F32, False)
    out = nc.declare_dram_parameter("out_t", [NB, T, O], F32, True)

    with tile.TileContext(nc) as tc:
        with tc.tile_pool(name="wpool", bufs=1) as wp, \
             tc.tile_pool(name="upool", bufs=1) as up, \
             tc.tile_pool(name="tmpp", bufs=1) as tp, \
             tc.tile_pool(name="xp", bufs=5) as xp, \
             tc.tile_pool(name="yp", bufs=4) as yp, \
             tc.tile_pool(name="stp", bufs=2) as stp, \
             tc.tile_pool(name="ofp", bufs=4) as ofp, \
             tc.tile_pool(name="ps_mm1", bufs=2, space="PSUM") as ps1, \
             tc.tile_pool(name="ps_y", bufs=2, space="PSUM") as psy, \
             tc.tile_pool(name="ps_st", bufs=1, space="PSUM") as pst, \
             tc.tile_pool(name="ps_p4", bufs=2, space="PSUM") as ps4:

            # ---- early weights (mm1 path) ----
            brg_t = wp.tile([128, 4 * 512], BF16, tag="brg")
            big_t = wp.tile([128, 4 * 512], BF16, tag="big")
            cst_t = wp.tile([128, 4 * 3], F32, tag="cst")
            for (dst, src) in ((brg_t, Brg), (big_t, Big)):
                nc.sync.dma_start(
                    out=dst[:].rearrange("p (k n) -> p k n", k=4),
                    in_=src[:].rearrange("(k p) n -> p k n", p=128))
            nc.sync.dma_start(out=cst_t[:], in_=CST[:])

            cr_t = wp.tile([128, 4 * 512], BF16, tag="cr")
            ci_t = wp.tile([128, 4 * 512], BF16, tag="ci")
            w12_t = wp.tile([128, 4 * 512], BF16, tag="w12")
            tab_t = wp.tile([128, 8 * T], BF16, tag="tabs")
            rho_t = wp.tile([128, 4 * SCH], F32, tag="rho")
            ones_t = wp.tile([128, 128], BF16, tag="ones")
            ones32 = wp.tile([1, 1], F32, tag="ones32")
            eps_t = wp.tile([128, 1], F32, tag="eps")
            nc.vector.memset(ones_t[:], 1.0)
            nc.vector.memset(ones32[:], 1.0)
            nc.vector.memset(eps_t[:], LN_EPS)

            def col(c, blk):
                return cst_t[:, c * 4 + blk:c * 4 + blk + 1]

            def ctab(bk):
                return tab_t[:, (2 * bk) * T:(2 * bk + 1) * T]

            def stab(bk):
                return tab_t[:, (2 * bk + 1) * T:(2 * bk + 2) * T]

            # u/h storage: per (b, Bk): R and I planes, token-contiguous
            U = up.tile([128, NB * 4 * 2 * T], BF16, tag="u")
            uv = U[:].rearrange("p (b k c t) -> p b k c t", b=NB, k=4, c=2)

            def uplane(b, bk, c):
                return uv[:, b:b + 1, bk:bk + 1, c:c + 1, :].squeeze()

            tmp1 = tp.tile([128, T], BF16, tag="tmp1")
            tmp2 = tp.tile([128, T], BF16, tag="tmp2")
            tmp3 = tp.tile([128, T], BF16, tag="tmp3")
            gmp1 = tp.tile([128, SCH], BF16, tag="gmp1")
            gmp2 = tp.tile([128, SCH], BF16, tag="gmp2")
            gmp3 = tp.tile([128, SCH], BF16, tag="gmp3")
            carry = tp.tile([128, 8], BF16, tag="carry")

            def mm1(b):
                xts = []
                for mt in range(NMT):
                    t0 = mt * 512
                    xt = xp.tile([128, 4 * 512], BF16, tag="xt")
                    nc.sync.dma_start(
                        out=xt[:].rearrange("p (k t) -> p k t", k=4),
                        in_=xin[b, :, t0:t0 + 512].rearrange("(k p) t -> p k t", p=128))
                    xts.append(xt)
                for ob in range(4):
                    for mt in range(NMT):
                        t0 = mt * 512
                        for (wt, c, bcol) in ((brg_t, 0, C_BRG), (big_t, 1, C_BIG)):
                            pm = ps1.tile([128, 512], F32, tag="pm1")
                            for kt in range(4):
                                nc.tensor.matmul(
                                    pm[:], wt[:, kt * 512 + ob * 128:kt * 512 + ob * 128 + 128],
                                    xts[mt][:, kt * 512:(kt + 1) * 512],
                                    start=(kt == 0), stop=(kt == 3))
                            nc.scalar.activation(
                                uplane(b, ob, c)[:, t0:t0 + 512], pm[:],
                                ACTF.Identity, bias=col(bcol, ob), scale=1.0)

            def unit_rotate(b, bk):
                # rotate full row: v = e^{-i theta s} * u   (in place)
                uR = uplane(b, bk, 0)
                uI = uplane(b, bk, 1)
                c_, s_ = ctab(bk), stab(bk)
                nc.vector.tensor_tensor(tmp1[:], c_, uR, ALU.mult)
                nc.vector.tensor_tensor(tmp2[:], s_, uR, ALU.mult)
                nc.vector.tensor_tensor(tmp3[:], s_, uI, ALU.mult)
                nc.vector.tensor_tensor(uR, tmp1[:], tmp3[:], ALU.add)
                nc.vector.tensor_tensor(tmp1[:], c_, uI, ALU.mult)
                nc.vector.tensor_tensor(uI, tmp1[:], tmp2[:], ALU.subtract)

            def unit_half(b, bk, hf, unrot=None):
                uR = uplane(b, bk, 0)
                uI = uplane(b, bk, 1)
                rho = rho_t[:, bk * SCH:(bk + 1) * SCH]
                s0 = hf * SCH
                sl = slice(s0, s0 + SCH)
                c_, s_ = ctab(bk)[:, sl], stab(bk)[:, sl]
                # scan: w = cumsum with decay rho (in place)
                for ci, pl in ((0, uR), (1, uI)):
                    ini = 0.0 if hf == 0 else carry[:, bk * 2 + ci:bk * 2 + ci + 1]
                    nc.vector.tensor_tensor_scan(
                        pl[:, sl], rho, pl[:, sl], ini, ALU.mult, ALU.add)
                if hf == 0:
                    # save chunk-boundary state before in-place unrotate
                    nc.vector.tensor_copy(carry[:, bk * 2:bk * 2 + 1],
                                          uR[:, s0 + SCH - 1:s0 + SCH])
                    nc.vector.tensor_copy(carry[:, bk * 2 + 1:bk * 2 + 2],
                                          uI[:, s0 + SCH - 1:s0 + SCH])
                # unrotate: h = e^{i theta t} * w   (in place)
                if unrot is None:
                    eng, t1, t2, t3 = nc.vector, tmp1, tmp2, tmp3
                else:
                    eng, t1, t2, t3 = unrot, gmp1, gmp2, gmp3
                eng.tensor_tensor(t1[:, :SCH], c_, uR[:, sl], ALU.mult)
                eng.tensor_tensor(t2[:, :SCH], s_, uR[:, sl], ALU.mult)
                eng.tensor_tensor(t3[:, :SCH], s_, uI[:, sl], ALU.mult)
                eng.tensor_tensor(uR[:, sl], t1[:, :SCH], t3[:, :SCH], ALU.subtract)
                eng.tensor_tensor(t1[:, :SCH], c_, uI[:, sl], ALU.mult)
                eng.tensor_tensor(uI[:, sl], t2[:, :SCH], t1[:, :SCH], ALU.add)

            def phase2_mt(b, mt):
                t0 = mt * 512
                ys = []
                y2s = []
                for ob in range(4):
                    p2 = psy.tile([128, 512], F32, tag="py")
                    for bk in range(4):
                        nc.tensor.matmul(
                            p2[:], cr_t[:, bk * 512 + ob * 128:bk * 512 + ob * 128 + 128],
                            uplane(b, bk, 0)[:, t0:t0 + 512],
                            start=(bk == 0), stop=False)
                    for bk in range(4):
                        nc.tensor.matmul(
                            p2[:], ci_t[:, bk * 512 + ob * 128:bk * 512 + ob * 128 + 128],
                            uplane(b, bk, 1)[:, t0:t0 + 512],
                            start=False, stop=(bk == 3))
                    y = yp.tile([128, 512], BF16, tag="y", name=f"y{ob}")
                    y2 = yp.tile([128, 512], BF16, tag="y2", name=f"y2_{ob}")
                    nc.scalar.activation(y[:], p2[:], ACTF.Identity,
                                         bias=col(C_CRCI, ob), scale=1.0)
                    nc.scalar.activation(y2[:], p2[:], ACTF.Square,
                                         bias=col(C_CRCI, ob), scale=1.0)
                    ys.append(y)
                    y2s.append(y2)
                # per-token stats [1, 512]
                s1 = pst.tile([1, 512], F32, tag="s1", name="s1")
                s2 = pst.tile([1, 512], F32, tag="s2", name="s2")
                for ob in range(4):
                    nc.tensor.matmul(s1[:], ones_t[:, 0:1], ys[ob][:],
                                     start=(ob == 0), stop=(ob == 3))
                for ob in range(4):
                    nc.tensor.matmul(s2[:], ones_t[:, 0:1], y2s[ob][:],
                                     start=(ob == 0), stop=(ob == 3))
                mean = stp.tile([1, 512], F32, tag="mean")
                ms = stp.tile([1, 512], F32, tag="ms")
                var = stp.tile([1, 512], F32, tag="var")
                sd = stp.tile([1, 512], F32, tag="sd")
                A1 = stp.tile([1, 512], F32, tag="A1")
                A1t = stp.tile([128, 4], F32, tag="A1t")
                nc.scalar.activation(mean[:], s1[:], ACTF.Copy, scale=1.0 / H)
                nc.scalar.activation(ms[:], mean[:], ACTF.Square)
                nc.vector.scalar_tensor_tensor(var[:], s2[:], 1.0 / H, ms[:],
                                               ALU.mult, ALU.subtract)
                nc.scalar.activation(sd[:], var[:], ACTF.Sqrt, bias=eps_t[0:1, :])
                nc.vector.reciprocal_approx_fast(A1[:], sd[:])
                # transpose A1 [1,512] -> [128,4] via PE: col tb = A1-slice^T @ [1]
                pa = ps4.tile([128, 512], F32, tag="p4", name="pa1t")
                for tb in range(4):
                    nc.tensor.matmul(pa[:, tb:tb + 1],
                                     A1[:, tb * 128:(tb + 1) * 128],
                                     ones32[:], start=True, stop=True)
                nc.scalar.activation(A1t[:], pa[:, 0:4], ACTF.Copy)
                # MLP collapsed + LN fold: p4t[t, o] = sum_k y[k,t] * W12c[k,o]
                for tb in range(4):
                    p4 = ps4.tile([128, 512], F32, tag="p4")
                    for kt in range(4):
                        nc.tensor.matmul(
                            p4[:], ys[kt][:, tb * 128:(tb + 1) * 128],
                            w12_t[:, kt * 512:(kt + 1) * 512],
                            start=(kt == 0), stop=(kt == 3))
                    outf = ofp.tile([128, 512], F32, tag="outf")
                    nc.scalar.activation(outf[:], p4[:], ACTF.Copy,
                                         scale=A1t[:, tb:tb + 1])
                    nc.sync.dma_start(
                        out=out[b, t0 + tb * 128:t0 + (tb + 1) * 128, :],
                        in_=outf[:])

            # ---- emission order (pipelining) ----
            mm1(0)
            # tables after mm1(0) DMAs, per-bk so bk0's tables land first
            for bk in range(4):
                nc.sync.dma_start(
                    out=tab_t[:, 2 * bk * T:(2 * bk + 2) * T].rearrange(
                        "p (g t) -> p g t", g=2),
                    in_=TABS[2 * bk * 128:(2 * bk + 2) * 128, :].rearrange(
                        "(g p) t -> p g t", p=128))
                nc.sync.dma_start(
                    out=rho_t[:, bk * SCH:(bk + 1) * SCH],
                    in_=RHO[bk * 128:(bk + 1) * 128, :])
            for bk in range(4):
                unit_rotate(0, bk)
                unit_half(0, bk, 0)
            mm1(1)
            for (dst, src) in ((cr_t, Crt), (ci_t, Cin), (w12_t, W12)):
                nc.sync.dma_start(
                    out=dst[:].rearrange("p (k n) -> p k n", k=4),
                    in_=src[:].rearrange("(k p) n -> p k n", p=128))
            for bk in range(4):
                unit_half(0, bk, 1)
            # b1 interleaved with phase2(0)
            for k in range(4):
                unit_rotate(1, k)
                unit_half(1, k, 0)
                phase2_mt(0, k)
            unit_half(1, 0, 1)
            phase2_mt(1, 0)
            unit_half(1, 1, 1)
            phase2_mt(1, 1)
            unit_half(1, 2, 1)
            unit_half(1, 3, 1)
            phase2_mt(1, 2)
            phase2_mt(1, 3)

    nc.compile()
    return nc


def _consts(nu_log, theta_log, gamma_log, br, bi, cr, ci, ln_scale, ln_bias,
            W1, b1, W2, b2):
    nu = np.exp(nu_log.astype(np.float64))
    theta = np.exp(theta_log.astype(np.float64))
    rho = np.exp(-nu)                       # |lambda|
    gamma = np.exp(gamma_log.astype(np.float64))
    W1s = W1.astype(np.float64) * ln_scale.astype(np.float64)[:, None]
    W12 = W1s @ W2.astype(np.float64)
    col6 = W12.sum(0)                        # ln_scale @ W1 @ W2
    # fold -mean*col6 into the weights: W12c = W12 - ones*col6/H
    W12c = W12 - col6[None, :] / H
    cols7 = ((ln_bias.astype(np.float64) @ W1.astype(np.float64)
              + b1.astype(np.float64)) @ W2.astype(np.float64)
             + b2.astype(np.float64)).astype(np.float32)
    cols = {}
    cols[C_BRG] = br.astype(np.float64) * gamma
    cols[C_BIG] = bi.astype(np.float64) * gamma
    cols[C_CRCI] = (cr - ci).astype(np.float64)
    cst = np.zeros((128, 4 * 3), np.float32)
    for c, v in cols.items():
        for blk in range(4):
            cst[:, c * 4 + blk] = v[blk * 128:(blk + 1) * 128].astype(np.float32)
    # twiddle tables: per Bk block, cos/sin(theta_h * t), [8*128, T]
    t_idx = np.arange(T, dtype=np.float64)
    ang = theta[:, None] * t_idx[None, :]          # [H, T]
    bf = ml_dtypes.bfloat16
    tabs = np.zeros((8 * 128, T), bf)
    for blk in range(4):
        hs = slice(blk * 128, (blk + 1) * 128)
        tabs[2 * blk * 128:(2 * blk + 1) * 128] = np.cos(ang[hs]).astype(bf)
        tabs[(2 * blk + 1) * 128:(2 * blk + 2) * 128] = np.sin(ang[hs]).astype(bf)
    rho_tab = np.repeat(rho.astype(np.float32)[:, None], SCH, axis=1)  # [512, SCH]
    return cst, tabs, rho_tab, gamma, W12c, cols7


def kernel(x, nu_log, theta_log, gamma_log, Br, br, Bi, bi,
           Cr, cr, Ci, ci, ln_scale, ln_bias, W1, b1, W2, b2):
    if "nc" not in _CACHE:
        _CACHE["nc"] = _build()
    nc = _CACHE["nc"]
    cst, tabs, rho_tab, gamma, W12c, cols7 = _consts(
        nu_log, theta_log, gamma_log, br, bi, cr, ci,
        ln_scale, ln_bias, W1, b1, W2, b2)
    bf = ml_dtypes.bfloat16
    g32 = gamma.astype(np.float32)
    Brg = (Br * g32[None, :]).astype(bf)
    Big = (Bi * g32[None, :]).astype(bf)
    Crb = Cr.astype(bf)
    Cinb = (-Ci).astype(bf)
    W12b = W12c.astype(np.float32).astype(bf)
    xt = np.ascontiguousarray(x.transpose(0, 2, 1)).astype(bf)  # [B, H, T]
    in_maps = []
    for i in range(NCORES):
        in_maps.append(dict(x_t=xt[2 * i:2 * i + 2], Brg=Brg, Big=Big,
                            Crt=Crb, Cin=Cinb, W12=W12b, tabs=tabs,
                            rho=rho_tab, cst=cst))
    res = run_bass_kernel_spmd(nc, in_maps, core_ids=list(range(NCORES)))
    out = np.empty((B, T, O), np.float32)
    for i in range(NCORES):
        out[2 * i:2 * i + 2] = res.results[i]["out_t"]  # [NB, T, O]
    if np.any(cols7):
        out += cols7[None, None, :]
    return out
